# revision 7
# baseline (speedup 1.0000x reference)
"""Trainium2 Bass kernel for the DSConv1d block (relu -> BN(eval) -> depthwise
conv1d(k=3,pad=1) -> PReLU -> GlobalLayerNorm -> pointwise conv -> residual).

Sharding: data-parallel over batch B=16 across 8 NeuronCores (2 samples/core).
Everything per-sample is device-local; no collectives.

All matmuls in bf16 (PE streams 1 col/cycle warm at bf16 vs 2 for fp32r).
Layout per core (2 samples of [512, 4000] f32):

  startup:  constants packed host-side into 3 DMA descriptors (dg [P,1536],
            wt [P,2048], small [P,16]); x tiles DMA'd in column halves
            ([0:2001], [2001:4000]) so the first conv group starts after
            ~1MB instead of ~10MB.
  phase 1:  per (ci,h): relu on ACT into bf16 g tiles (halo cols on DVE);
            depthwise conv as 3 PSUM-accumulated diagonal matmuls (BN scale
            folded into diag weights, BN shift via pad cols + PReLU bias);
            PReLU on ACT with fused per-partition sum (accum_out);
            sum(p^2) via one DVE scalar_tensor_tensor with accum_out.
  stats:    cross-partition reduce via ones-matmul, scalar chain, rstd
            folded into the bf16 pointwise weights (one [P,2048] scale op).
  phase 2:  pointwise conv as K=512 GEMM (4x128 k-tiles) on the PE;
            epilogue (psum + d + x residual) in one DVE stt; y DMA out.

  The two samples are software-pipelined: sample b+1's depthwise groups are
  emitted between sample b's stats and pointwise groups so the PE never
  idles >3us (keeps the HAM clock-gate at K=8/8 = 2.4 GHz).
"""

import numpy as np
import ml_dtypes

B, C, T = 16, 512, 4000
NCORES = 8
BPC = B // NCORES          # samples per core
CT = 4                     # channel tiles of 128
P = 128
TH = 2                     # halves of T
HW_ = T // TH              # 2000
CHUNKS = [(0, 512), (512, 512), (1024, 512), (1536, 464)]  # bank-aligned
BN_EPS = 1e-5
GLN_EPS = 1e-8
XSPLIT = HW_ + 1           # x DMA half boundary (2001): covers h=0 halo

_CACHE = {}


def _build(alpha: float):
    import concourse.bass as bass
    import concourse.mybir as mybir
    import concourse.tile as tile
    from concourse import bacc

    dt = mybir.dt.float32
    db = mybir.dt.bfloat16
    AF = mybir.ActivationFunctionType
    OP = mybir.AluOpType

    nc = bacc.Bacc("TRN2", target_bir_lowering=False, debug=False)

    x_d = nc.dram_tensor("x", [BPC, C, T], dt, kind="ExternalInput")
    dg_d = nc.dram_tensor("dg", [P, CT * 3 * P], db, kind="ExternalInput")
    wt_d = nc.dram_tensor("wt", [P, CT * C], db, kind="ExternalInput")
    small_d = nc.dram_tensor("small", [P, 16], dt, kind="ExternalInput")
    y_d = nc.dram_tensor("y", [BPC, C, T], dt, kind="ExternalOutput")

    with tile.TileContext(nc) as tc:
        with (
            tc.tile_pool(name="cpool", bufs=1) as cpool,
            tc.tile_pool(name="xpool", bufs=6) as xpool,
            tc.tile_pool(name="ppool", bufs=8) as ppool,
            tc.tile_pool(name="gpool", bufs=4) as gpool,
            tc.tile_pool(name="opool", bufs=2) as opool,
            tc.tile_pool(name="wscp", bufs=1) as wscp,
            tc.tile_pool(name="spool", bufs=2) as spool,
            tc.tile_pool(name="pspool", bufs=2, space=bass.MemorySpace.PSUM) as pspool,
        ):
            # ---- constants: 3 batched descriptors ----
            dgt = cpool.tile([P, CT * 3 * P], db, tag="dg")
            nc.sync.dma_start(dgt[:], dg_d[:])
            small = cpool.tile([P, 16], dt, tag="small")
            wtt = cpool.tile([P, CT * C], db, tag="wt")
            ones = cpool.tile([P, 1], dt, tag="ones")
            nc.vector.memset(ones[:], 1.0)

            xt = {}
            pt = {}
            sums = {}

            def diag(ci, k):
                return dgt[:, (ci * 3 + k) * P:(ci * 3 + k + 1) * P]

            def load_x(b, ci):
                t = xpool.tile([P, T], dt, tag="x", name=f"x{b}_{ci}")
                nc.sync.dma_start(t[:, 0:XSPLIT],
                                  x_d[b, ci * P:(ci + 1) * P, 0:XSPLIT])
                nc.sync.dma_start(t[:, XSPLIT:T],
                                  x_d[b, ci * P:(ci + 1) * P, XSPLIT:T])
                xt[(b, ci)] = t

            def dw_group(b, ci, h):
                """relu + depthwise conv + prelu + sum(p^2) for one
                [128, 2000] tile."""
                idx = ci * TH + h
                o0 = h * HW_
                g = gpool.tile([P, HW_ + 2], db, tag="g")
                nc.scalar.activation(
                    g[:, 1:HW_ + 1], xt[(b, ci)][:, o0:o0 + HW_], AF.Relu)
                if h == 0:
                    nc.vector.tensor_copy(g[:, 0:1], small[:, ci:ci + 1])
                    nc.vector.tensor_scalar_max(
                        g[:, HW_ + 1:HW_ + 2],
                        xt[(b, ci)][:, o0 + HW_:o0 + HW_ + 1], 0.0)
                else:
                    nc.vector.tensor_scalar_max(
                        g[:, 0:1], xt[(b, ci)][:, o0 - 1:o0], 0.0)
                    nc.vector.tensor_copy(
                        g[:, HW_ + 1:HW_ + 2], small[:, ci:ci + 1])

                cps = pspool.tile([P, 2048], dt, tag="ps")
                for k in range(3):
                    for c0, wc in CHUNKS:
                        nc.tensor.matmul(
                            cps[:, c0:c0 + wc], diag(ci, k),
                            g[:, k + c0: k + c0 + wc],
                            start=(k == 0), stop=(k == 2))
                # PReLU(conv + bsum) with fused per-partition sum
                nc.scalar.activation(
                    pt[(b, ci)][:, o0:o0 + HW_], cps[:, 0:HW_], AF.Prelu,
                    bias=small[:, 4 + ci:5 + ci], scale=1.0, alpha=alpha,
                    accum_out=sums[b][:, idx:idx + 1])
                # sum(p^2): (p*1)*p with running accumulator, one DVE op
                sq = gpool.tile([P, HW_ + 2], db, tag="g")
                nc.vector.scalar_tensor_tensor(
                    sq[:, 0:HW_], pt[(b, ci)][:, o0:o0 + HW_], 1.0,
                    pt[(b, ci)][:, o0:o0 + HW_], OP.mult, OP.mult,
                    accum_out=sums[b][:, 8 + idx:9 + idx])

            def dw_ci(b, ci):
                if ci == 0:
                    pt[(b, ci)] = ppool.tile([P, T], db, tag="p",
                                             name=f"pt{b}_{ci}")
                    sums[b] = spool.tile([P, 16], dt, tag="sums",
                                         name=f"sums{b}")
                else:
                    pt[(b, ci)] = ppool.tile([P, T], db, tag="p",
                                             name=f"pt{b}_{ci}")
                dw_group(b, ci, 0)
                dw_group(b, ci, 1)

            stats_out = {}

            def emit_stats(b):
                spr = pspool.tile([1, 16], dt, tag="ps")
                nc.tensor.matmul(spr[0:1, :], ones[:], sums[b][:], start=True,
                                 stop=True)
                st = spool.tile([1, 16], dt, tag="st")
                iS, iQ, iMEAN, iE2, iMSQ, iVAR, iA, iS0, iR0, iAR, iS1, \
                    iRSTD, iRM = range(13)

                def stc(i):
                    return st[0:1, i:i + 1]

                nc.vector.tensor_reduce(stc(iS), spr[0:1, 0:8],
                                        mybir.AxisListType.X, OP.add)
                nc.vector.tensor_reduce(stc(iQ), spr[0:1, 8:16],
                                        mybir.AxisListType.X, OP.add)
                invN = 1.0 / float(C * T)
                nc.vector.tensor_scalar_mul(stc(iMEAN), stc(iS), invN)
                nc.vector.tensor_scalar_mul(stc(iE2), stc(iQ), invN)
                nc.vector.tensor_scalar(stc(iMSQ), stc(iMEAN), stc(iMEAN),
                                        None, OP.mult)
                nc.vector.scalar_tensor_tensor(stc(iVAR), stc(iMSQ), -1.0,
                                               stc(iE2), OP.mult, OP.add)
                nc.vector.tensor_scalar_add(stc(iA), stc(iVAR), GLN_EPS)
                nc.scalar.activation(stc(iS0), stc(iA), AF.Sqrt)
                nc.vector.reciprocal(stc(iR0), stc(iS0))
                # one Newton step for sqrt: s1 = 0.5*(s0 + a*r0)
                nc.vector.tensor_scalar(stc(iAR), stc(iA), stc(iR0), None,
                                        OP.mult)
                nc.vector.tensor_scalar(stc(iS1), stc(iAR), stc(iS0), 0.5,
                                        OP.add, OP.mult)
                nc.vector.reciprocal(stc(iRSTD), stc(iS1))
                nc.vector.tensor_scalar(stc(iRM), stc(iRSTD), stc(iMEAN),
                                        -1.0, OP.mult, OP.mult)
                rstd_b = spool.tile([P, 1], dt, tag="rstd_b")
                rm_b = spool.tile([P, 1], dt, tag="rm_b")
                nc.gpsimd.partition_broadcast(rstd_b[:], stc(iRSTD))
                nc.gpsimd.partition_broadcast(rm_b[:], stc(iRM))
                d = spool.tile([P, CT], dt, tag="d")
                nc.vector.scalar_tensor_tensor(d[:], small[:, 8:12],
                                               rm_b[:, 0:1], small[:, 12:16],
                                               OP.mult, OP.add)
                wsc = wscp.tile([P, CT * C], db, tag="wsc")
                nc.vector.tensor_scalar_mul(wsc[:], wtt[:], rstd_b[:, 0:1])
                stats_out[b] = (wsc, d)

            def pw_group(b, oi, h):
                wsc, d = stats_out[b]
                o0 = h * HW_
                ops = pspool.tile([P, 2048], dt, tag="ps")
                for k in range(CT):
                    for c0, wc in CHUNKS:
                        nc.tensor.matmul(
                            ops[:, c0:c0 + wc],
                            wsc[:, k * C + oi * P: k * C + (oi + 1) * P],
                            pt[(b, k)][:, o0 + c0: o0 + c0 + wc],
                            start=(k == 0), stop=(k == CT - 1))
                ot = opool.tile([P, HW_], dt, tag="o")
                nc.vector.scalar_tensor_tensor(
                    ot[:], ops[:, 0:HW_], d[:, oi:oi + 1],
                    xt[(b, oi)][:, o0:o0 + HW_], OP.add, OP.add)
                nc.sync.dma_start(
                    y_d[b, oi * P:(oi + 1) * P, o0:o0 + HW_], ot[:])

            # ---------------- emission schedule ----------------
            # startup: dg first (needed by first conv), x(b0,ci0), smalls,
            # then the rest of x(b0) interleaved with wt.
            load_x(0, 0)
            nc.sync.dma_start(small[:], small_d[:])
            load_x(0, 1)
            nc.sync.dma_start(wtt[:], wt_d[:])
            load_x(0, 2)
            load_x(0, 3)

            for ci in range(CT):
                dw_ci(0, ci)

            for b in range(BPC):
                nb = b + 1
                if nb < BPC:
                    for ci in range(CT):
                        load_x(nb, ci)
                    dw_ci(nb, 0)
                emit_stats(b)
                if nb < BPC:
                    dw_ci(nb, 1)
                for oi in range(3):
                    pw_group(b, oi, 0)
                    pw_group(b, oi, 1)
                if nb < BPC:
                    dw_ci(nb, 2)
                    dw_ci(nb, 3)
                pw_group(b, 3, 0)
                pw_group(b, 3, 1)

    nc.compile()
    return nc


def _host_prep(bn_gamma, bn_beta, bn_mean, bn_var, dw_w, gln_gamma, gln_beta,
               pw_w):
    f64 = np.float64
    bf16 = ml_dtypes.bfloat16
    s = bn_gamma.astype(f64) / np.sqrt(bn_var.astype(f64) + BN_EPS)
    bb = bn_beta.astype(f64) - bn_mean.astype(f64) * s
    w = dw_w[:, 0, :].astype(f64)                      # [C, 3]
    dg = np.zeros((P, CT * 3 * P), bf16)
    for ci in range(CT):
        sl = slice(ci * P, (ci + 1) * P)
        for k in range(3):
            j0 = (ci * 3 + k) * P
            dg[:, j0:j0 + P] = np.diag((s[sl] * w[sl, k])).astype(bf16)
    s_safe = np.where(np.abs(s) < 1e-12, 1e-12, s)
    pads = (-bb / s_safe).reshape(CT, P).T                     # [P, CT]
    bsum = (bb * w.sum(1)).reshape(CT, P).T
    wtT = (pw_w.astype(f64) * gln_gamma.astype(f64)[None, :]).T   # [C, O]
    wt = np.ascontiguousarray(
        wtT.reshape(CT, P, C).transpose(1, 0, 2).reshape(P, CT * C)
    ).astype(bf16)
    wgam = (pw_w.astype(f64) @ gln_gamma.astype(f64)).reshape(CT, P).T
    wbet = (pw_w.astype(f64) @ gln_beta.astype(f64)).reshape(CT, P).T
    small = np.concatenate([pads, bsum, wgam, wbet], axis=1) \
        .astype(np.float32).copy()                             # [P, 16]
    return dg, wt, small


def _get_program(alpha: float, fp32r: bool = True):
    key = round(float(alpha), 9)
    if key not in _CACHE:
        _CACHE[key] = _build(float(alpha))
    return _CACHE[key]


def run(inputs: dict, trace: bool = False, fp32r: bool = True):
    """Run on 8 cores; returns (y_full, BassKernelResults)."""
    from concourse.bass_utils import run_bass_kernel_spmd

    inputs = {k: np.asarray(v) for k, v in inputs.items()}
    x = np.ascontiguousarray(inputs["x"], dtype=np.float32)
    alpha = float(np.asarray(inputs["prelu_a"]).reshape(-1)[0])
    dg, wt, small = _host_prep(
        inputs["bn_gamma"], inputs["bn_beta"], inputs["bn_mean"],
        inputs["bn_var"], inputs["dw_w"], inputs["gln_gamma"],
        inputs["gln_beta"], inputs["pw_w"])
    nc = _get_program(alpha)
    consts = dict(dg=dg, wt=wt, small=small)
    in_maps = [
        {"x": x[i * BPC:(i + 1) * BPC], **consts} for i in range(NCORES)
    ]
    res = run_bass_kernel_spmd(nc, in_maps, list(range(NCORES)), trace=trace)
    y = np.concatenate([res.results[i]["y"] for i in range(NCORES)], axis=0)
    return y, res


def kernel(**inputs) -> np.ndarray:
    y, _ = run(inputs, trace=False)
    return y


# revision 10
# speedup vs baseline: 1.6283x; 1.6283x over previous
"""Trainium2 Bass kernel for the DSConv1d block (relu -> BN(eval) -> depthwise
conv1d(k=3,pad=1) -> PReLU -> GlobalLayerNorm -> pointwise conv -> residual).

Sharding: data-parallel over batch B=16 across 8 NeuronCores (2 samples/core).
Everything per-sample is device-local; no collectives.

All matmuls in bf16 (PE streams 1 col/cycle warm at bf16 vs 2 for fp32r).
Layout per core (2 samples of [512, 4000] f32):

  startup:  constants packed host-side into 3 DMA descriptors (dg [P,1536],
            wt [P,2048], small [P,16]); x tiles DMA'd in column halves
            ([0:2001], [2001:4000]) so the first conv group starts after
            ~1MB instead of ~10MB.
  phase 1:  per (ci,h): relu on ACT into bf16 g tiles (halo cols on DVE);
            depthwise conv as 3 PSUM-accumulated diagonal matmuls (BN scale
            folded into diag weights, BN shift via pad cols + PReLU bias);
            PReLU on ACT with fused per-partition sum (accum_out);
            sum(p^2) via one DVE scalar_tensor_tensor with accum_out.
  stats:    cross-partition reduce via ones-matmul, scalar chain, rstd
            folded into the bf16 pointwise weights (one [P,2048] scale op).
  phase 2:  pointwise conv as K=512 GEMM (4x128 k-tiles) on the PE;
            epilogue (psum + d + x residual) in one DVE stt; y DMA out.

  The two samples are software-pipelined: sample b+1's depthwise groups are
  emitted between sample b's stats and pointwise groups so the PE never
  idles >3us (keeps the HAM clock-gate at K=8/8 = 2.4 GHz).
"""

import numpy as np
import ml_dtypes

B, C, T = 16, 512, 4000
NCORES = 8
BPC = B // NCORES          # samples per core
CT = 4                     # channel tiles of 128
P = 128
TH = 2                     # halves of T
HW_ = T // TH              # 2000
CHUNKS = [(0, 512), (512, 512), (1024, 512), (1536, 464)]  # bank-aligned
BN_EPS = 1e-5
GLN_EPS = 1e-8
XSPLIT = HW_ + 1           # x DMA half boundary (2001): covers h=0 halo

_CACHE = {}


def _build(alpha: float):
    import concourse.bass as bass
    import concourse.mybir as mybir
    import concourse.tile as tile
    from concourse import bacc

    dt = mybir.dt.float32
    db = mybir.dt.bfloat16
    AF = mybir.ActivationFunctionType
    OP = mybir.AluOpType

    nc = bacc.Bacc("TRN2", target_bir_lowering=False, debug=False)

    x_d = nc.dram_tensor("x", [BPC, C, T], dt, kind="ExternalInput")
    dg_d = nc.dram_tensor("dg", [P, CT * 3 * P], db, kind="ExternalInput")
    wt_d = nc.dram_tensor("wt", [P, CT * C], db, kind="ExternalInput")
    small_d = nc.dram_tensor("small", [P, 16], dt, kind="ExternalInput")
    y_d = nc.dram_tensor("y", [BPC, C, T], dt, kind="ExternalOutput")

    with tile.TileContext(nc) as tc:
        with (
            tc.tile_pool(name="cpool", bufs=1) as cpool,
            tc.tile_pool(name="xpool", bufs=6) as xpool,
            tc.tile_pool(name="ppool", bufs=8) as ppool,
            tc.tile_pool(name="gpool", bufs=4) as gpool,
            tc.tile_pool(name="opool", bufs=2) as opool,
            tc.tile_pool(name="wscp", bufs=1) as wscp,
            tc.tile_pool(name="spool", bufs=2) as spool,
            tc.tile_pool(name="pspool", bufs=2, space=bass.MemorySpace.PSUM) as pspool,
        ):
            # ---- constants: 3 batched descriptors ----
            dgt = cpool.tile([P, CT * 3 * P], db, tag="dg")
            nc.sync.dma_start(dgt[:], dg_d[:])
            small = cpool.tile([P, 16], dt, tag="small")
            wtt = cpool.tile([P, CT * C], db, tag="wt")
            ones = cpool.tile([P, 1], dt, tag="ones")
            nc.vector.memset(ones[:], 1.0)

            xt = {}
            pt = {}
            sums = {}

            def diag(ci, k):
                return dgt[:, (ci * 3 + k) * P:(ci * 3 + k + 1) * P]

            def load_x(b, ci):
                t = xpool.tile([P, T], dt, tag="x", name=f"x{b}_{ci}")
                nc.sync.dma_start(t[:, 0:XSPLIT],
                                  x_d[b, ci * P:(ci + 1) * P, 0:XSPLIT])
                nc.sync.dma_start(t[:, XSPLIT:T],
                                  x_d[b, ci * P:(ci + 1) * P, XSPLIT:T])
                xt[(b, ci)] = t

            gt = {}

            def emit_relu(b, ci, h):
                """relu into a fresh bf16 g tile (one dw-group ahead so the
                ACT FIFO never stalls the PE's conv groups)."""
                o0 = h * HW_
                g = gpool.tile([P, HW_ + 2], db, tag="g",
                               name=f"g{b}_{ci}_{h}")
                nc.scalar.activation(
                    g[:, 1:HW_ + 1], xt[(b, ci)][:, o0:o0 + HW_], AF.Relu)
                if h == 0:
                    nc.vector.tensor_copy(g[:, 0:1], small[:, ci:ci + 1])
                    nc.vector.tensor_scalar_max(
                        g[:, HW_ + 1:HW_ + 2],
                        xt[(b, ci)][:, o0 + HW_:o0 + HW_ + 1], 0.0)
                else:
                    nc.vector.tensor_scalar_max(
                        g[:, 0:1], xt[(b, ci)][:, o0 - 1:o0], 0.0)
                    nc.vector.tensor_copy(
                        g[:, HW_ + 1:HW_ + 2], small[:, ci:ci + 1])
                gt[(b, ci, h)] = g

            def dw_group(b, ci, h):
                """depthwise conv + prelu + sum(p^2) for one [128, 2000]
                tile (g produced earlier by emit_relu)."""
                idx = ci * TH + h
                o0 = h * HW_
                if h == 0 and ci == 0:
                    sums[b] = spool.tile([P, 16], dt, tag="sums",
                                         name=f"sums{b}")
                if h == 0:
                    pt[(b, ci)] = ppool.tile([P, T], db, tag="p",
                                             name=f"pt{b}_{ci}")
                g = gt.pop((b, ci, h))
                cps = pspool.tile([P, 2048], dt, tag="ps")
                for k in range(3):
                    for c0, wc in CHUNKS:
                        nc.tensor.matmul(
                            cps[:, c0:c0 + wc], diag(ci, k),
                            g[:, k + c0: k + c0 + wc],
                            start=(k == 0), stop=(k == 2))
                # PReLU(conv + bsum) with fused per-partition sum
                nc.scalar.activation(
                    pt[(b, ci)][:, o0:o0 + HW_], cps[:, 0:HW_], AF.Prelu,
                    bias=small[:, 4 + ci:5 + ci], scale=1.0, alpha=alpha,
                    accum_out=sums[b][:, idx:idx + 1])
                # sum(p^2): (p*1)*p with running accumulator, one DVE op.
                # Output scribbles over the now-dead g tile (no extra SBUF).
                nc.vector.scalar_tensor_tensor(
                    g[:, 0:HW_], pt[(b, ci)][:, o0:o0 + HW_], 1.0,
                    pt[(b, ci)][:, o0:o0 + HW_], OP.mult, OP.mult,
                    accum_out=sums[b][:, 8 + idx:9 + idx])

            stats_out = {}

            def emit_stats(b):
                spr = pspool.tile([1, 16], dt, tag="ps")
                nc.tensor.matmul(spr[0:1, :], ones[:], sums[b][:], start=True,
                                 stop=True)
                st = spool.tile([1, 16], dt, tag="st")
                iS, iQ, iMEAN, iE2, iMSQ, iVAR, iA, iS0, iR0, iAR, iS1, \
                    iRSTD, iRM = range(13)

                def stc(i):
                    return st[0:1, i:i + 1]

                nc.vector.tensor_reduce(stc(iS), spr[0:1, 0:8],
                                        mybir.AxisListType.X, OP.add)
                nc.vector.tensor_reduce(stc(iQ), spr[0:1, 8:16],
                                        mybir.AxisListType.X, OP.add)
                invN = 1.0 / float(C * T)
                nc.vector.tensor_scalar_mul(stc(iMEAN), stc(iS), invN)
                nc.vector.tensor_scalar_mul(stc(iE2), stc(iQ), invN)
                nc.vector.tensor_scalar(stc(iMSQ), stc(iMEAN), stc(iMEAN),
                                        None, OP.mult)
                nc.vector.scalar_tensor_tensor(stc(iVAR), stc(iMSQ), -1.0,
                                               stc(iE2), OP.mult, OP.add)
                nc.vector.tensor_scalar_add(stc(iA), stc(iVAR), GLN_EPS)
                nc.scalar.activation(stc(iS0), stc(iA), AF.Sqrt)
                nc.vector.reciprocal(stc(iR0), stc(iS0))
                # one Newton step for sqrt: s1 = 0.5*(s0 + a*r0)
                nc.vector.tensor_scalar(stc(iAR), stc(iA), stc(iR0), None,
                                        OP.mult)
                nc.vector.tensor_scalar(stc(iS1), stc(iAR), stc(iS0), 0.5,
                                        OP.add, OP.mult)
                nc.vector.reciprocal(stc(iRSTD), stc(iS1))
                nc.vector.tensor_scalar(stc(iRM), stc(iRSTD), stc(iMEAN),
                                        -1.0, OP.mult, OP.mult)
                rstd_b = spool.tile([P, 1], dt, tag="rstd_b")
                rm_b = spool.tile([P, 1], dt, tag="rm_b")
                nc.gpsimd.partition_broadcast(rstd_b[:], stc(iRSTD))
                nc.gpsimd.partition_broadcast(rm_b[:], stc(iRM))
                d = spool.tile([P, CT], dt, tag="d")
                nc.vector.scalar_tensor_tensor(d[:], small[:, 8:12],
                                               rm_b[:, 0:1], small[:, 12:16],
                                               OP.mult, OP.add)
                wsc = wscp.tile([P, CT * C], db, tag="wsc")
                nc.vector.tensor_scalar_mul(wsc[:], wtt[:], rstd_b[:, 0:1])
                stats_out[b] = (wsc, d)

            def pw_group(b, oi, h):
                wsc, d = stats_out[b]
                o0 = h * HW_
                ops = pspool.tile([P, 2048], dt, tag="ps")
                for k in range(CT):
                    for c0, wc in CHUNKS:
                        nc.tensor.matmul(
                            ops[:, c0:c0 + wc],
                            wsc[:, k * C + oi * P: k * C + (oi + 1) * P],
                            pt[(b, k)][:, o0 + c0: o0 + c0 + wc],
                            start=(k == 0), stop=(k == CT - 1))
                ot = opool.tile([P, HW_], dt, tag="o")
                nc.vector.scalar_tensor_tensor(
                    ot[:], ops[:, 0:HW_], d[:, oi:oi + 1],
                    xt[(b, oi)][:, o0:o0 + HW_], OP.add, OP.add)
                nc.sync.dma_start(
                    y_d[b, oi * P:(oi + 1) * P, o0:o0 + HW_], ot[:])

            # ---------------- emission schedule ----------------
            # startup: dg first (needed by first conv), x(b0,ci0), smalls,
            # then the rest of x(b0) interleaved with wt.
            load_x(0, 0)
            nc.sync.dma_start(small[:], small_d[:])
            load_x(0, 1)
            nc.sync.dma_start(wtt[:], wt_d[:])
            load_x(0, 2)
            load_x(0, 3)

            # global dw-group order; each group's relu is emitted one group
            # ahead of its conv so the ACT FIFO (relu, prelu alternating)
            # never gates the PE.
            dw_seq = [(b, ci, h) for b in range(BPC)
                      for ci in range(CT) for h in range(TH)]
            dw_pos = 0

            def dw_next():
                nonlocal dw_pos
                grp = dw_seq[dw_pos]
                if dw_pos + 1 < len(dw_seq):
                    emit_relu(*dw_seq[dw_pos + 1])
                dw_group(*grp)
                dw_pos += 1

            emit_relu(*dw_seq[0])
            for _ in range(2 * CT - 2):        # phase 1 of sample 0
                dw_next()
            if BPC > 1:
                for ci in range(CT):           # prefetch x of sample 1 before
                    load_x(1, ci)              # its first relu is emitted
            dw_next()
            dw_next()

            for b in range(BPC):
                nb = b + 1
                if nb < BPC:
                    dw_next()                  # dw(nb, ci0, h0)
                    dw_next()                  # dw(nb, ci0, h1)
                    emit_stats(b)
                    pw_group(b, 0, 0)
                    pw_group(b, 0, 1)
                    # alternate remaining 6 dw groups with pw groups
                    dw_next(); pw_group(b, 1, 0)
                    dw_next(); pw_group(b, 1, 1)
                    dw_next(); pw_group(b, 2, 0)
                    dw_next(); pw_group(b, 2, 1)
                    dw_next(); pw_group(b, 3, 0)
                    dw_next()
                    emit_stats(nb)
                    pw_group(b, 3, 1)
                else:
                    for oi in range(CT):
                        pw_group(b, oi, 0)
                        pw_group(b, oi, 1)

    nc.compile()
    return nc


def _host_prep(bn_gamma, bn_beta, bn_mean, bn_var, dw_w, gln_gamma, gln_beta,
               pw_w):
    f64 = np.float64
    bf16 = ml_dtypes.bfloat16
    s = bn_gamma.astype(f64) / np.sqrt(bn_var.astype(f64) + BN_EPS)
    bb = bn_beta.astype(f64) - bn_mean.astype(f64) * s
    w = dw_w[:, 0, :].astype(f64)                      # [C, 3]
    dg = np.zeros((P, CT * 3 * P), bf16)
    for ci in range(CT):
        sl = slice(ci * P, (ci + 1) * P)
        for k in range(3):
            j0 = (ci * 3 + k) * P
            dg[:, j0:j0 + P] = np.diag((s[sl] * w[sl, k])).astype(bf16)
    s_safe = np.where(np.abs(s) < 1e-12, 1e-12, s)
    pads = (-bb / s_safe).reshape(CT, P).T                     # [P, CT]
    bsum = (bb * w.sum(1)).reshape(CT, P).T
    wtT = (pw_w.astype(f64) * gln_gamma.astype(f64)[None, :]).T   # [C, O]
    wt = np.ascontiguousarray(
        wtT.reshape(CT, P, C).transpose(1, 0, 2).reshape(P, CT * C)
    ).astype(bf16)
    wgam = (pw_w.astype(f64) @ gln_gamma.astype(f64)).reshape(CT, P).T
    wbet = (pw_w.astype(f64) @ gln_beta.astype(f64)).reshape(CT, P).T
    small = np.concatenate([pads, bsum, wgam, wbet], axis=1) \
        .astype(np.float32).copy()                             # [P, 16]
    return dg, wt, small


def _get_program(alpha: float, fp32r: bool = True):
    key = round(float(alpha), 9)
    if key not in _CACHE:
        _CACHE[key] = _build(float(alpha))
    return _CACHE[key]


def run(inputs: dict, trace: bool = False, fp32r: bool = True):
    """Run on 8 cores; returns (y_full, BassKernelResults)."""
    from concourse.bass_utils import run_bass_kernel_spmd

    inputs = {k: np.asarray(v) for k, v in inputs.items()}
    x = np.ascontiguousarray(inputs["x"], dtype=np.float32)
    alpha = float(np.asarray(inputs["prelu_a"]).reshape(-1)[0])
    dg, wt, small = _host_prep(
        inputs["bn_gamma"], inputs["bn_beta"], inputs["bn_mean"],
        inputs["bn_var"], inputs["dw_w"], inputs["gln_gamma"],
        inputs["gln_beta"], inputs["pw_w"])
    nc = _get_program(alpha)
    consts = dict(dg=dg, wt=wt, small=small)
    in_maps = [
        {"x": x[i * BPC:(i + 1) * BPC], **consts} for i in range(NCORES)
    ]
    res = run_bass_kernel_spmd(nc, in_maps, list(range(NCORES)), trace=trace)
    y = np.concatenate([res.results[i]["y"] for i in range(NCORES)], axis=0)
    return y, res


def kernel(**inputs) -> np.ndarray:
    y, _ = run(inputs, trace=False)
    return y
